# revision 15
# baseline (speedup 1.0000x reference)
"""Trainium2 Bass kernel for nn_CategoryBranch2 (3 convs + 2 BiGRUs).

Distribution: conv is data-parallel (core c = sample c). The GRU scans are
resharded so each core scans ONE direction for TWO samples (its pair
partner's sample gathered via a pair AllGather): even cores run fwd, odd
cores bwd. Both samples share every recurrent-weight LDWEIGHTS, halving the
per-core PE instruction stream that dominates scan time.

The SPMD program is identical on all cores; direction differences enter as
data: per-core wi/wh/bias inputs hold only that core's direction, and a
parity vector drives an arithmetic select between normal and time-reversed
copies of the shared inputs (bwd cores scan time-reversed gx).

Phases per core:
  1  conv blocks (identical to v1) -> yp[128, 32, 1024] bf16 in DRAM.
  1g pair AllGather yp -> ypg[2, ...].
  2  ysel = par0*ypg[s] + par1*reverse_t(ypg[s]) built per k-quarter;
     gx1[s] = wi1(my dir) @ ysel[s] + bias, fp32 [128, 12, 1024] per sample.
  3  L1 scan, 1024 steps, n=2 (both samples in one matmul per block).
  3g pair AllGather of louts1 -> l2in[s] = fwd louts + bwd louts (symmetric).
  4  l2sel = parity select+reverse of l2in; gx2; L2 scan, 512 steps, n=2.
  5  out = my-direction louts2 for both samples; host sums pair outputs.

Self-contained: hardcodes all shapes; host does numpy weight re-layouts.
"""

import numpy as np
import ml_dtypes

import concourse.bacc as bacc
import concourse.bass as bass
import concourse.mybir as mybir
from concourse.tile import TileContext
from concourse.bass import ds
from concourse.bass_utils import run_bass_kernel_spmd

BF16 = ml_dtypes.bfloat16
F32 = mybir.dt.float32
BF = mybir.dt.bfloat16
AF = mybir.ActivationFunctionType
OP = mybir.AluOpType
PE = mybir.EngineType.PE

BN_EPS = 1e-5
GROUPS = [[0, 1], [2, 3], [4, 5], [6, 7]]

X9_LEN = 134 * 130          # 17420
X2_LEN = 132 * 66 + 66      # 8778
X3_LEN = 130 * 34 + 34      # 4454
YPST_LEN = 32 * 128         # 4096

_CACHED_NC = {}
_SKIP_MM = False
_SKIP_TAIL = False


def _scan_superstep(nc, vs, ps, wh, gxv, bhn, h_both, louts, tg, s_out):
    """One time step of one direction for BOTH samples.
    h_both [128, 8] bf16: cols kc*2+s. gxv = (rz_lo [128,6,2,T],
    rz_hi [128,2,2,T], n [128,4,2,T]) sample-interleaved fp32 views."""
    psum = ps.tile([128, 8, 2], F32, tag="scan_psum", name="psum")
    psn = ps.tile([128, 4, 2], F32, tag="scan_psn", name="psn")
    nc.vector.tensor_copy(psum[:, 0:6, :, None], gxv[0][:, :, :, ds(tg, 1)])
    nc.vector.tensor_copy(psum[:, 6:8, :, None], gxv[1][:, :, :, ds(tg, 1)])
    nc.vector.tensor_copy(psn[:, :, :],
                          bhn[:, :, None].to_broadcast((128, 4, 2)))
    if not _SKIP_MM:
        for khalf in range(2):
            for mc in range(12):
                dst = psum[:, mc, :] if mc < 8 else psn[:, mc - 8, :]
                for kc in (2 * khalf, 2 * khalf + 1):
                    nc.tensor.matmul(
                        dst,
                        wh[:, (kc * 12 + mc) * 128:(kc * 12 + mc + 1) * 128],
                        h_both[:, kc * 2:kc * 2 + 2],
                        start=False, stop=(kc == 3), skip_group_check=True)
    if _SKIP_TAIL:
        return
    rz = vs.tile([128, 16], F32, tag="rz")
    nc.scalar.activation(rz[:].rearrange("p (g j) -> p g j", g=2),
                         psum[:, :, :].rearrange("p (g m) c -> p g (m c)", g=2),
                         AF.Sigmoid)
    zc = vs.tile([128, 8], F32, tag="zc")
    nc.gpsimd.tensor_scalar(zc, rz[:, 8:16], -1.0, 1.0, OP.mult, OP.add)
    q = vs.tile([128, 8], F32, tag="q")
    nc.vector.tensor_tensor(q[:].rearrange("p (m c) -> p m c", c=2),
                            rz[:, 0:8].rearrange("p (m c) -> p m c", c=2),
                            psn[:, :, :], OP.mult)
    npre = vs.tile([128, 8], F32, tag="npre")
    q2 = q[:].rearrange("p (m c) -> p m c", c=2)
    npre2 = npre[:].rearrange("p (m c) -> p m c", c=2)
    nc.vector.tensor_tensor(npre2[:, :, :, None], q2[:, :, :, None],
                            gxv[2][:, :, :, ds(tg, 1)], OP.add)
    n = vs.tile([128, 8], BF, tag="n")
    nc.scalar.activation(n, npre, AF.Tanh)
    a = vs.tile([128, 8], F32, tag="a")
    nc.gpsimd.tensor_mul(a, rz[:, 8:16], h_both)
    b = vs.tile([128, 8], F32, tag="b")
    nc.vector.tensor_tensor(b, zc, n, OP.mult)
    f = vs.tile([128, 8], F32, tag="f")
    nc.vector.tensor_tensor(f, a, b, OP.add)
    nc.scalar.activation(h_both[:, 0:4], f[:, 0:4], AF.Tanh)
    nc.scalar.activation(h_both[:, 4:8], f[:, 4:8], AF.Tanh)
    if s_out is not None:
        hb2 = h_both[:].rearrange("p (k c) -> p k c", c=2)
        for s in range(2):
            nc.vector.tensor_copy(louts[s][:, :, ds(s_out, 1)],
                                  hb2[:, :, s, None])


def _scan_blk(nc, vs, ps, base, sbase, wh, gxv, bhn, h_both, louts, BLK):
    for pi in range(BLK // 2):
        for par in range(2):
            t_loc = pi * 2 + par
            _scan_superstep(nc, vs, ps, wh, gxv, bhn, h_both, louts,
                            base + t_loc, (sbase + pi) if par == 0 else None)


def _scan_loop(nc, tc, vs, ps, nblk, wh, gxv, bhn, h_both, louts,
               timing_wrap=False, BLK=32, static=False):
    if nblk == 0:
        return
    if static:
        for blki in range(nblk):
            _scan_blk(nc, vs, ps, blki * BLK, blki * (BLK // 2), wh, gxv,
                      bhn, h_both, louts, BLK)
        return
    with tc.For_i(0, nblk, 1, hint_engines=(PE,)) as blk:
        if timing_wrap:
            base = blk * 0
            sbase = blk * 0
        else:
            base = blk * BLK
            sbase = blk * (BLK // 2)
        _scan_blk(nc, vs, ps, base, sbase, wh, gxv, bhn, h_both, louts, BLK)


def build_nc(nblk1=32, nblk2=16, timing_wrap=False, skip_mm=False, skip_tail=False, static=False):
    global _SKIP_MM, _SKIP_TAIL
    _SKIP_MM, _SKIP_TAIL = skip_mm, skip_tail
    nc = bacc.Bacc("TRN2", target_bir_lowering=False, debug=False, num_devices=8)

    # ---------------- inputs ----------------
    xp_d = nc.dram_tensor("xp", [1031 * 130], BF, kind="ExternalInput")
    w1s_d = nc.dram_tensor("w1s", [128, 64], BF, kind="ExternalInput")
    w2s_d = nc.dram_tensor("w2s", [9, 128, 128], BF, kind="ExternalInput")
    w3s_d = nc.dram_tensor("w3s", [9, 128, 2, 128], BF, kind="ExternalInput")
    cb1_d = nc.dram_tensor("cb1", [64, 1], F32, kind="ExternalInput")
    sc1_d = nc.dram_tensor("sc1", [64, 1], F32, kind="ExternalInput")
    sh1_d = nc.dram_tensor("sh1", [64, 1], F32, kind="ExternalInput")
    cb2_d = nc.dram_tensor("cb2", [128, 1], F32, kind="ExternalInput")
    sc2_d = nc.dram_tensor("sc2", [128, 1], F32, kind="ExternalInput")
    sh2_d = nc.dram_tensor("sh2", [128, 1], F32, kind="ExternalInput")
    cb3_d = nc.dram_tensor("cb3", [128, 2], F32, kind="ExternalInput")
    sc3_d = nc.dram_tensor("sc3", [128, 2], F32, kind="ExternalInput")
    sh3_d = nc.dram_tensor("sh3", [128, 2], F32, kind="ExternalInput")
    par_d = nc.dram_tensor("par", [128, 2], F32, kind="ExternalInput")
    # my-direction GRU weights
    wi1_d = nc.dram_tensor("wi1", [12, 128, 32, 128], BF, kind="ExternalInput")
    gxb1_d = nc.dram_tensor("gxb1", [128, 12], F32, kind="ExternalInput")
    wh1_d = nc.dram_tensor("wh1", [128, 4 * 12 * 128], BF, kind="ExternalInput")
    bhn1_d = nc.dram_tensor("bhn1", [128, 4], F32, kind="ExternalInput")
    wi2_d = nc.dram_tensor("wi2", [128, 12 * 4 * 128], BF, kind="ExternalInput")
    gxb2_d = nc.dram_tensor("gxb2", [128, 12], F32, kind="ExternalInput")
    wh2_d = nc.dram_tensor("wh2", [128, 4 * 12 * 128], BF, kind="ExternalInput")
    bhn2_d = nc.dram_tensor("bhn2", [128, 4], F32, kind="ExternalInput")

    out_d = nc.dram_tensor("out", [128, 2, 4, 256], F32, kind="ExternalOutput")
    yp_d = nc.dram_tensor("yp", [128, 32, 1024], BF, kind="Internal")
    ypg_d = nc.dram_tensor("ypg", [2, 128, 32, 1024], BF, kind="Internal")
    lo1_d = nc.dram_tensor("lo1", [128, 4, 2, 512], BF, kind="Internal")
    log_d = nc.dram_tensor("log", [2, 128, 4, 2, 512], BF, kind="Internal")

    with TileContext(nc) as tc:
      with tc.tile_pool(name="keep", bufs=1) as keep, \
           tc.tile_pool(name="arena1", bufs=1) as ar1, \
           tc.tile_pool(name="arena2", bufs=1) as ar2, \
           tc.tile_pool(name="scan_vs", bufs=4) as vs, \
           tc.tile_pool(name="scan_ps", bufs=2, space="PSUM") as ps:
        # ---- long-lived small tiles
        par = keep.tile([128, 2], F32, tag="par")
        nc.sync.dma_start(par, par_d[:])
        gxb1 = keep.tile([128, 12], F32, tag="gxb1")
        nc.sync.dma_start(gxb1, gxb1_d[:])
        bhn1 = keep.tile([128, 4], F32, tag="bhn1")
        nc.sync.dma_start(bhn1, bhn1_d[:])
        gxb2 = keep.tile([128, 12], F32, tag="gxb2")
        nc.sync.dma_start(gxb2, gxb2_d[:])
        bhn2 = keep.tile([128, 4], F32, tag="bhn2")
        nc.sync.dma_start(bhn2, bhn2_d[:])
        louts1 = []
        louts2 = []
        for s in range(2):
            lo = keep.tile([128, 4, 512], BF, tag=f"lo1_{s}", name=f"lo1_{s}")
            nc.vector.memset(lo[:], 0.0)
            louts1.append(lo)
            lo2 = keep.tile([128, 4, 256], F32, tag=f"lo2_{s}", name=f"lo2_{s}")
            nc.vector.memset(lo2[:], 0.0)
            louts2.append(lo2)
        l2in = keep.tile([128, 4, 2, 512], BF, tag="l2in")
        l2sel = keep.tile([128, 2, 4, 512], BF, tag="l2sel")
        h1_both = keep.tile([128, 8], BF, tag="h1b")
        nc.vector.memset(h1_both[:], 0.0)
        h2_both = keep.tile([128, 8], BF, tag="h2b")
        nc.vector.memset(h2_both[:], 0.0)

        # ================== phase 1: convs (same as v1) ==================
        a1c = ar1.tile([128, X9_LEN + X3_LEN], BF, tag="ar1")
        x9 = a1c[:, 0:X9_LEN]
        x3 = a1c[:, X9_LEN:X9_LEN + X3_LEN]
        a2c = ar2.tile([128, X2_LEN + YPST_LEN], BF, tag="ar2")
        x2 = a2c[:, 0:X2_LEN]
        ypst = a2c[:, X2_LEN:X2_LEN + YPST_LEN]
        nc.vector.memset(a1c[:], 0.0)
        nc.vector.memset(a2c[:], 0.0)

        with tc.tile_pool(name="cw", bufs=1) as cw, \
             tc.tile_pool(name="p1psum", bufs=4, space="PSUM") as pp1, \
             tc.tile_pool(name="p1tmp", bufs=3) as tp1:
            w1s = cw.tile([128, 64], BF)
            nc.sync.dma_start(w1s, w1s_d[:])
            w2s = cw.tile([128, 9 * 128], BF)
            nc.sync.dma_start(w2s[:].rearrange("p (s j) -> p s j", s=9),
                              w2s_d[:].rearrange("s p j -> p s j"))
            w3s = cw.tile([128, 9 * 2 * 128], BF)
            nc.sync.dma_start(
                w3s[:].rearrange("p (s c j) -> p s c j", s=9, c=2),
                w3s_d[:].rearrange("s p c j -> p s c j"))
            cb1 = cw.tile([64, 1], F32)
            nc.sync.dma_start(cb1, cb1_d[:])
            sc1 = cw.tile([64, 1], F32)
            nc.sync.dma_start(sc1, sc1_d[:])
            sh1 = cw.tile([64, 1], F32)
            nc.sync.dma_start(sh1, sh1_d[:])
            cb2 = cw.tile([128, 1], F32)
            nc.sync.dma_start(cb2, cb2_d[:])
            sc2 = cw.tile([128, 1], F32)
            nc.sync.dma_start(sc2, sc2_d[:])
            sh2 = cw.tile([128, 1], F32)
            nc.sync.dma_start(sh2, sh2_d[:])
            cb3 = cw.tile([128, 2], F32)
            nc.sync.dma_start(cb3, cb3_d[:])
            sc3 = cw.tile([128, 2], F32)
            nc.sync.dma_start(sc3, sc3_d[:])
            sh3 = cw.tile([128, 2], F32)
            nc.sync.dma_start(sh3, sh3_d[:])

            for i in range(8):
                t0 = i * 128
                for dh in range(3):
                    for dw in range(3):
                        s = dh * 3 + dw
                        start = (t0 + dh) * 130 + dw
                        nc.sync.dma_start(x9[s:s + 1, 0:132 * 130],
                                          xp_d[ds(start, 132 * 130)][None, :])
                # ---- conv1: 33 chunks of (4 rows x 128 f)
                for c in range(33):
                    psum = pp1.tile([128, 512], F32, tag="cpsum")
                    rhs = x9[:, c * 520:c * 520 + 520].rearrange(
                        "p (r w) -> p r w", w=130)[:, :, 0:128]
                    nc.tensor.matmul(psum[0:64], w1s, rhs, start=True, stop=True)
                    tmp = tp1.tile([64, 512], BF, tag="c1tmp")
                    nc.scalar.activation(tmp, psum[0:64], AF.Relu, bias=cb1)
                    tr = tmp[:].rearrange("q (r f e) -> q r f e", f=64, e=2)
                    pm = tp1.tile([64, 256], BF, tag="c1pm")
                    pmr = pm[:].rearrange("q (r f) -> q r f", f=64)
                    nc.vector.tensor_tensor(pmr, tr[:, :, :, 0], tr[:, :, :, 1],
                                            OP.max)
                    xv = x2[0:64, c * 264:c * 264 + 264].rearrange(
                        "q (r w) -> q r w", w=66)[:, :, 1:65]
                    nc.vector.scalar_tensor_tensor(
                        xv, pmr, sc1, sh1[:, 0:1, None].to_broadcast(pmr.shape),
                        OP.mult, OP.add)
                if i == 0:
                    nc.vector.memset(x2[0:64, 0:132], 0.0)
                if i == 7:
                    nc.vector.memset(x2[0:64, 130 * 66:132 * 66], 0.0)
                # ---- conv2: 17 chunks of (<=8 rows x 64 f)
                for c in range(17):
                    r0 = c * 8
                    rows = min(8, 130 - r0)
                    nfree = rows * 64
                    psum = pp1.tile([128, 512], F32, tag="cpsum")
                    for si in range(9):
                        dh, dw = si // 3, si % 3
                        off = (r0 + dh) * 66 + dw
                        rhs = x2[:, off:off + rows * 66].rearrange(
                            "p (r w) -> p r w", w=66)[:, :, 0:64]
                        nc.tensor.matmul(psum[:, 0:nfree],
                                         w2s[:, si * 128:(si + 1) * 128],
                                         rhs, start=(si == 0), stop=(si == 8))
                    tmp = tp1.tile([128, 512], BF, tag="c2tmp")
                    nc.scalar.activation(tmp[:, 0:nfree], psum[:, 0:nfree],
                                         AF.Relu, bias=cb2)
                    tr = tmp[:, 0:nfree].rearrange("p (r f e) -> p r f e",
                                                   f=32, e=2)
                    pm = tp1.tile([128, 256], BF, tag="c2pm")
                    pmr = pm[:, 0:rows * 32].rearrange("p (r f) -> p r f", f=32)
                    nc.vector.tensor_tensor(pmr, tr[:, :, :, 0], tr[:, :, :, 1],
                                            OP.max)
                    xv = x3[:, r0 * 34:r0 * 34 + rows * 34].rearrange(
                        "p (r w) -> p r w", w=34)[:, :, 1:33]
                    nc.vector.scalar_tensor_tensor(
                        xv, pmr, sc2, sh2[:, 0:1, None].to_broadcast(pmr.shape),
                        OP.mult, OP.add)
                if i == 0:
                    nc.vector.memset(x3[:, 0:34], 0.0)
                if i == 7:
                    nc.vector.memset(x3[:, 129 * 34:130 * 34], 0.0)
                # ---- conv3: 2 co-chunks x 8 chunks of (16 rows x 32 f)
                for ch in range(2):
                    for c in range(8):
                        r0 = c * 16
                        psum = pp1.tile([128, 512], F32, tag="cpsum")
                        for si in range(9):
                            dh, dw = si // 3, si % 3
                            off = (r0 + dh) * 34 + dw
                            rhs = x3[:, off:off + 16 * 34].rearrange(
                                "p (r w) -> p r w", w=34)[:, :, 0:32]
                            nc.tensor.matmul(
                                psum,
                                w3s[:, (si * 2 + ch) * 128:(si * 2 + ch + 1) * 128],
                                rhs, start=(si == 0), stop=(si == 8))
                        tmp = tp1.tile([128, 512], BF, tag="c3tmp")
                        nc.scalar.activation(tmp, psum, AF.Relu,
                                             bias=cb3[:, ch:ch + 1])
                        tr = tmp[:].rearrange("p (r f e) -> p f r e", f=16, e=2)
                        pm = tp1.tile([128, 256], BF, tag="c3pm")
                        pmr = pm[:].rearrange("p (f r) -> p f r", r=16)
                        nc.vector.tensor_tensor(pmr, tr[:, :, :, 0],
                                                tr[:, :, :, 1], OP.max)
                        yv = ypst[:].rearrange("p (f c t) -> p f c t",
                                               f=16, c=2)[:, :, ch, r0:r0 + 16]
                        nc.vector.scalar_tensor_tensor(
                            yv, pmr, sc3[:, ch:ch + 1],
                            sh3[:, ch:ch + 1, None].to_broadcast(pmr.shape),
                            OP.mult, OP.add)
                nc.sync.dma_start(yp_d[:, :, ds(t0, 128)],
                                  ypst[:].rearrange("p (k t) -> p k t", k=32))

        # ================== phase 1g: gather pair yp ==================
        nc.gpsimd.collective_compute(
            "AllGather", OP.bypass, replica_groups=GROUPS,
            ins=[yp_d[:]], outs=[ypg_d[:]])

        # ================== phase 2: gx1 for both samples ==================
        gx_t0 = ar1.tile([128, 12288], F32, tag="ar1")
        gx_t1 = ar2.tile([128, 12288], F32, tag="ar2")
        gx_lo = gx_t0[:].rearrange("p (m s t) -> p m s t", m=6, s=2)
        gx_hi = gx_t1[:].rearrange("p (m s t) -> p m s t", m=6, s=2)
        gxv1 = (gx_lo, gx_hi[:, 0:2], gx_hi[:, 2:6])
        with tc.tile_pool(name="ysel", bufs=1) as yselp, \
             tc.tile_pool(name="ytmp", bufs=2) as ytmpp, \
             tc.tile_pool(name="wi1sb", bufs=2) as wip, \
             tc.tile_pool(name="p2psum", bufs=4, space="PSUM") as pp2:
            for quar in range(4):
                ysels = []
                for s in range(2):
                    ysel = yselp.tile([128, 8, 1024], BF, tag=f"ysel{s}",
                                      name=f"ysel{s}")
                    ytmp = ytmpp.tile([128, 8, 1024], BF, tag="ytmp")
                    nc.sync.dma_start(ytmp, ypg_d[s, :, ds(quar * 8, 8), :])
                    nc.vector.tensor_scalar_mul(ysel, ytmp, par[:, 0:1])
                    rev = ytmp[:, :, 1023::-1]
                    nc.vector.scalar_tensor_tensor(
                        ysel, rev[:, :, 0:1024], par[:, 1:2], ysel,
                        OP.mult, OP.add)
                    ysels.append(ysel)
                for mc in range(12):
                    wisb = wip.tile([128, 8 * 128], BF, tag="wi1t")
                    nc.sync.dma_start(
                        wisb[:].rearrange("p (k j) -> p k j", k=8),
                        wi1_d[mc, :, ds(quar * 8, 8), :])
                    for s in range(2):
                        for tch in range(2):
                            psum = pp2.tile([128, 512], F32, tag="gxpsum")
                            for kc in range(8):
                                nc.tensor.matmul(
                                    psum, wisb[:, kc * 128:(kc + 1) * 128],
                                    ysels[s][:, kc, ds(tch * 512, 512)],
                                    start=(kc == 0), stop=(kc == 7))
                            if mc < 6:
                                gview = gx_lo[:, mc, s, tch * 512:(tch + 1) * 512]
                            else:
                                gview = gx_hi[:, mc - 6, s, tch * 512:(tch + 1) * 512]
                            if quar == 0:
                                nc.vector.tensor_scalar_add(
                                    gview, psum, gxb1[:, mc:mc + 1])
                            else:
                                nc.vector.tensor_tensor(gview, gview, psum,
                                                        OP.add)

        # ================== phase 3: L1 scan ==================
        with tc.tile_pool(name="l1w", bufs=1) as l1w:
            wh1sb = l1w.tile([128, 4 * 12 * 128], BF, tag="wh1")
            nc.sync.dma_start(wh1sb, wh1_d[:])
            _scan_loop(nc, tc, vs, ps, nblk1, wh1sb, gxv1, bhn1, h1_both,
                       louts1, timing_wrap, static=static)

        # ================== phase 3g: exchange louts1 ==================
        for s in range(2):
            nc.sync.dma_start(lo1_d[:, :, s, :], louts1[s])
        nc.gpsimd.collective_compute(
            "AllGather", OP.bypass, replica_groups=GROUPS,
            ins=[lo1_d[:]], outs=[log_d[:]])
        lg0 = keep.tile([128, 4, 2, 512], BF, tag="lg0")
        nc.sync.dma_start(lg0, log_d[0])
        nc.sync.dma_start(l2in, log_d[1])
        nc.vector.tensor_tensor(l2in, l2in, lg0, OP.add)
        # parity select + reverse -> l2sel[s] [128, 4, 512]
        for s in range(2):
            nc.vector.tensor_scalar_mul(l2sel[:, s], l2in[:, :, s, :],
                                        par[:, 0:1])
            rev = l2in[:, :, s, 511::-1]
            nc.vector.scalar_tensor_tensor(
                l2sel[:, s], rev[:, :, 0:512], par[:, 1:2], l2sel[:, s],
                OP.mult, OP.add)

        # ================== phase 4: gx2 + L2 scan ==================
        gx2_t = ar1.tile([128, 12288], F32, tag="ar1")
        gx2_i = gx2_t[:].rearrange("p (m s t) -> p m s t", m=12, s=2)
        gxv2 = (gx2_i[:, 0:6], gx2_i[:, 6:8], gx2_i[:, 8:12])
        w4 = ar2.tile([128, 12288], BF, tag="ar2")
        wi2sb = w4[:, 0:6144]
        wh2sb = w4[:, 6144:12288]
        nc.sync.dma_start(wi2sb, wi2_d[:])
        nc.sync.dma_start(wh2sb, wh2_d[:])
        with tc.tile_pool(name="p4psum", bufs=4, space="PSUM") as pp4:
            for s in range(2):
                for mc in range(12):
                    psum = pp4.tile([128, 512], F32, tag="gx2psum")
                    for kc in range(4):
                        nc.tensor.matmul(
                            psum,
                            wi2sb[:, (mc * 4 + kc) * 128:(mc * 4 + kc + 1) * 128],
                            l2sel[:, s, kc, :],
                            start=(kc == 0), stop=(kc == 3))
                    nc.vector.tensor_scalar_add(gx2_i[:, mc, s, :], psum,
                                                gxb2[:, mc:mc + 1])

            _scan_loop(nc, tc, vs, ps, nblk2, wh2sb, gxv2, bhn2, h2_both,
                       louts2, timing_wrap, static=static)

            for s in range(2):
                nc.sync.dma_start(out_d[:, s], louts2[s])

    nc.compile()
    return nc


# --------------------------------------------------------------------------
# host-side preprocessing
# --------------------------------------------------------------------------

def _bn(g, be, rm, rv):
    s = np.asarray(g) / np.sqrt(np.asarray(rv) + BN_EPS)
    return (s.astype(np.float32),
            (np.asarray(be) - np.asarray(rm) * s).astype(np.float32))


def _prep_conv(inputs):
    d = {}
    cw1 = np.asarray(inputs['cw1'])
    w1s = np.zeros((128, 64), np.float32)
    for dh in range(3):
        for dw in range(3):
            w1s[dh * 3 + dw] = cw1[:, 0, dh, dw]
    d['w1s'] = w1s.astype(BF16)
    w2 = np.asarray(inputs['cw2'])
    w2s = np.zeros((9, 128, 128), np.float32)
    w2s[:, 0:64, :] = w2.transpose(2, 3, 1, 0).reshape(9, 64, 128)
    d['w2s'] = w2s.astype(BF16)
    w3 = np.asarray(inputs['cw3'])
    d['w3s'] = np.ascontiguousarray(
        w3.transpose(2, 3, 1, 0).reshape(9, 128, 2, 128)).astype(BF16)
    sc1, sh1 = _bn(inputs['g1'], inputs['be1'], inputs['rm1'], inputs['rv1'])
    sc2, sh2 = _bn(inputs['g2'], inputs['be2'], inputs['rm2'], inputs['rv2'])
    sc3, sh3 = _bn(inputs['g3'], inputs['be3'], inputs['rm3'], inputs['rv3'])
    d['cb1'] = np.asarray(inputs['cb1'], np.float32).reshape(64, 1)
    d['sc1'] = sc1.reshape(64, 1)
    d['sh1'] = sh1.reshape(64, 1)
    d['cb2'] = np.asarray(inputs['cb2'], np.float32).reshape(128, 1)
    d['sc2'] = sc2.reshape(128, 1)
    d['sh2'] = sh2.reshape(128, 1)
    d['cb3'] = np.ascontiguousarray(
        np.asarray(inputs['cb3'], np.float32).reshape(2, 128).T)
    d['sc3'] = np.ascontiguousarray(sc3.reshape(2, 128).T)
    d['sh3'] = np.ascontiguousarray(sh3.reshape(2, 128).T)
    return d


def _prep_dir(inputs, tag):
    """GRU weights for one direction ('f' or 'b')."""
    d = {}
    dprime = np.arange(4096)
    perm = (dprime % 256) * 16 + dprime // 256

    wi = np.asarray(inputs[f'wi{tag}1'])[:, perm]
    d['wi1'] = np.ascontiguousarray(
        wi.reshape(12, 128, 32, 128).transpose(0, 3, 2, 1)).astype(BF16)
    bias = np.asarray(inputs[f'bi{tag}1']).copy()
    bh = np.asarray(inputs[f'bh{tag}1'])
    bias[:1024] += bh[:1024]
    d['gxb1'] = np.ascontiguousarray(bias.reshape(12, 128).T)
    d['wh1'] = np.asarray(inputs[f'wh{tag}1']).reshape(
        12, 128, 4, 128).transpose(3, 2, 0, 1).reshape(128, -1).astype(BF16)
    d['bhn1'] = np.ascontiguousarray(bh[1024:].reshape(4, 128).T)
    wi2v = np.asarray(inputs[f'wi{tag}2'])
    d['wi2'] = wi2v.reshape(12, 128, 4, 128).transpose(
        3, 0, 2, 1).reshape(128, -1).astype(BF16)
    bias2 = np.asarray(inputs[f'bi{tag}2']).copy()
    bh2 = np.asarray(inputs[f'bh{tag}2'])
    bias2[:1024] += bh2[:1024]
    d['gxb2'] = np.ascontiguousarray(bias2.reshape(12, 128).T)
    d['wh2'] = np.asarray(inputs[f'wh{tag}2']).reshape(
        12, 128, 4, 128).transpose(3, 2, 0, 1).reshape(128, -1).astype(BF16)
    d['bhn2'] = np.ascontiguousarray(bh2[1024:].reshape(4, 128).T)
    return d


def _prep_sample(x_c):
    xp = np.zeros((1031, 130), np.float32)
    xp[3:1027, 1:129] = x_c
    return {'xp': xp.astype(BF16).reshape(-1)}


def get_nc(nblk1=32, nblk2=16):
    key = (nblk1, nblk2)
    if key not in _CACHED_NC:
        _CACHED_NC[key] = build_nc(nblk1, nblk2)
    return _CACHED_NC[key]


def prep_in_maps(inputs):
    conv = _prep_conv(inputs)
    dirs = {'f': _prep_dir(inputs, 'f'), 'b': _prep_dir(inputs, 'b')}
    x = np.asarray(inputs['x'])
    in_maps = []
    for c in range(8):
        m = dict(conv)
        m.update(dirs['f' if c % 2 == 0 else 'b'])
        m.update(_prep_sample(x[c, 0]))
        parv = np.zeros((128, 2), np.float32)
        parv[:, c % 2] = 1.0
        m['par'] = parv
        in_maps.append(m)
    return in_maps


def run(inputs, nblk1=32, nblk2=16, **rkw):
    nc = get_nc(nblk1, nblk2)
    in_maps = prep_in_maps(inputs)
    return run_bass_kernel_spmd(nc, in_maps, core_ids=list(range(8)), **rkw)


def assemble(res):
    outs = []
    for a in range(4):
        oe = np.asarray(res.results[2 * a]['out'])     # fwd [128, 2, 4, 256]
        ob = np.asarray(res.results[2 * a + 1]['out'])  # bwd
        for s in range(2):
            o = oe[:, s] + ob[:, s]                    # [128, 4, 256]
            outs.append(np.ascontiguousarray(
                o.transpose(2, 1, 0).reshape(256, 512)))
    return np.stack(outs).astype(np.float32)


def kernel(**inputs) -> np.ndarray:
    return assemble(run(inputs))
